# revision 22
# baseline (speedup 1.0000x reference)
"""Trainium2 Bass kernel for the AttentionUnit GNN message-passing block.

Math
----
The nn.Module lifts scalars to `channel` dims with rank-1 weights, so the
whole block collapses to per-batch scalar attention:

    s[b,i,j] = alpha * e[b,i] * v[b,j],     alpha = w_g . w_f
    E = exp(s);  cs[j] = sum_i E[i,j];  rs[i] = sum_j E[i,j]
    out_v = v + beta  * E   @ (v / cs),     beta  = w_h . w_m
    out_e = e + gamma * E^T @ (e / rs),     gamma = w_l . w_n

Since |s| <= m ~ 0.3 (data-dependent, computed at runtime), exp(s) is
replaced by a degree-DEG Chebyshev-interpolated polynomial, which makes E a
rank-(DEG+1) matrix  E = sum_k c_k (e^k)(v^k)^T  that is never materialized:

    den = sum_k c_k A'_k X^k        (cs on the v-half, rs on the e-half)
    Y_k = sum_j X^{k+1} / den       (fused multiply-reduce against 1/den)
    out = swap(X) + sum_k g_k X^k   (g_k = swapped, scaled Y_k)

Layout: pure data parallel over 8 cores, 64 batch rows per core, stacked as
X = [v rows (partitions 0..63); e rows (64..127)] so every op handles both
sides at once. Cross-half swaps of [128,few] scalar blocks use two tiny
SBUF->SBUF DMAs; the final residual add uses a pre-swapped copy of the
input DMA'd at start, so no PE/transpose work is needed anywhere.

The polynomial coefficients depend on the input data, so they are passed as
small input tensors -> the compiled NEFF is input-independent and cached.
"""

import os
from contextlib import ExitStack

import numpy as np

import concourse.bass as bass
import concourse.tile as tile
from concourse import bacc, mybir
from concourse.bass_utils import run_bass_kernel_spmd

B = 512          # batch
D = 512          # dim
N_CORES = 8
BC = B // N_CORES  # 64 batch rows per core
P = 128            # partitions: [v (0..63); e (64..127)]
DEG = int(os.environ.get("ATTN_KERNEL_DEG", "4"))

f32 = mybir.dt.float32
MULT = mybir.AluOpType.mult
ADD = mybir.AluOpType.add
NCOL = 8  # padded column count for the R/Y scalar blocks
AF = mybir.ActivationFunctionType


def _build_program(deg: int):
    """Build + compile the single-core Tile program (same NEFF on all 8 cores)."""
    assert deg in (3, 4), "chains below are written for deg in {3, 4}"
    nc = bacc.Bacc(
        "TRN2",
        target_bir_lowering=False,
        debug=False,
        enable_asserts=False,
    )

    xv_d = nc.dram_tensor("xv", [BC, D], f32, kind="ExternalInput")
    xe_d = nc.dram_tensor("xe", [BC, D], f32, kind="ExternalInput")
    # coefs columns: [0] = c_0 * D
    #                [1 : deg+1]       = c_k (k=1..deg)            (den scale)
    #                [deg+1 : 2deg+2]  = swapped-cout * c_k (k=0..deg)
    coefs_d = nc.dram_tensor("coefs", [P, 2 * deg + 2], f32, kind="ExternalInput")
    ov_d = nc.dram_tensor("out_v", [BC, D], f32, kind="ExternalOutput")
    oe_d = nc.dram_tensor("out_e", [BC, D], f32, kind="ExternalOutput")

    with tile.TileContext(nc) as tc, ExitStack() as ctx:
        big = ctx.enter_context(tc.tile_pool(name="big", bufs=1))
        scr = ctx.enter_context(tc.tile_pool(name="scr", bufs=2))
        small = ctx.enter_context(tc.tile_pool(name="small", bufs=1))

        # ---- inputs: X and its half-swapped copy (for the final residual) --
        X = big.tile([P, D], f32, name="X")
        nc.sync.dma_start(X[0:BC, :], xv_d[:])
        nc.scalar.dma_start(X[BC:P, :], xe_d[:])
        Xs = big.tile([P, D], f32, name="Xs")
        nc.sync.dma_start(Xs[BC:P, :], xv_d[:])
        nc.scalar.dma_start(Xs[0:BC, :], xe_d[:])
        coefs = small.tile([P, 2 * deg + 2], f32, name="coefs_t")
        nc.gpsimd.dma_start(coefs[:], coefs_d[:])

        # ---- dual power chains: P_k = X^k (for the output sum) and
        # Ps_k = Xs^k (for the swapped denominator + Y reductions). The
        # row-sums R_k of the X powers are exactly the coefficients the
        # SWAPPED denominator needs, so no cross-half moves are required. --
        R1t = small.tile([P, 1], f32, name="R1t")
        nc.vector.tensor_reduce(R1t[:], X[:], axis=mybir.AxisListType.X, op=ADD)
        R2t = small.tile([P, 1], f32, name="R2t")
        P2 = big.tile([P, D], f32, name="P2")
        nc.scalar.activation(P2[:], X[:], AF.Square, accum_out=R2t[:])
        P2s = big.tile([P, D], f32, name="P2s")
        nc.scalar.activation(P2s[:], Xs[:], AF.Square)
        R3t = small.tile([P, 1], f32, name="R3t")
        P3 = big.tile([P, D], f32, name="P3")
        nc.vector.scalar_tensor_tensor(
            out=P3[:], in0=P2[:], scalar=1.0, in1=X[:],
            op0=MULT, op1=MULT, accum_out=R3t[:],
        )
        Rts = {1: R1t, 2: R2t, 3: R3t}
        Pw = {1: X, 2: P2, 3: P3}
        if deg >= 4:
            R4t = small.tile([P, 1], f32, name="R4t")
            P4 = big.tile([P, D], f32, name="P4")
            nc.scalar.activation(P4[:], P2[:], AF.Square, accum_out=R4t[:])
            Rts[4] = R4t
            Pw[4] = P4
        P3s = big.tile([P, D], f32, name="P3s")
        nc.vector.scalar_tensor_tensor(
            out=P3s[:], in0=P2s[:], scalar=1.0, in1=Xs[:], op0=MULT, op1=MULT,
        )
        Pws = {1: Xs, 2: P2s, 3: P3s}

        # b_k = c_k * R_k (tiny ACT copies with per-partition scale,
        # unblocking as each R lands; ACT is otherwise idle here)
        Bts = {}
        for k in range(1, deg + 1):
            Bts[k] = small.tile([P, 1], f32, name=f"B{k}t")
            nc.scalar.mul(Bts[k][:], coefs[:, k : k + 1], Rts[k][:])

        # ---- den_s = swap(den) = cd0 + sum_k b_k Xs^k ----
        dB = scr.tile([P, D], f32, name="dB", tag="t")
        nc.scalar.activation(dB[:], P2s[:], AF.Identity,
                             bias=coefs[:, 0:1], scale=Bts[2][:])
        dA = scr.tile([P, D], f32, name="dA", tag="u")
        nc.vector.scalar_tensor_tensor(
            out=dA[:], in0=Xs[:], scalar=Bts[1][:], in1=dB[:],
            op0=MULT, op1=ADD,
        )
        def emit_p4s():
            P4s = big.tile([P, D], f32, name="P4s")
            nc.scalar.activation(P4s[:], P2s[:], AF.Square)
            Pws[4] = P4s

        if deg >= 4:
            emit_p4s()
        dprev = dA
        for k in range(3, deg + 1):
            dnx = scr.tile([P, D], f32, name=f"d{k}", tag="t" if k % 2 else "u")
            nc.vector.scalar_tensor_tensor(
                out=dnx[:], in0=Pws[k][:], scalar=Bts[k][:], in1=dprev[:],
                op0=MULT, op1=ADD,
            )
            dprev = dnx
        den = dprev
        if deg == 3:
            # emitted late so the in-order ACT stream runs the B/dB ops
            # first (P4s is only consumed by the last Y reduction)
            emit_p4s()

        # ---- Y_ks = sum_j Xs^{k+1} / den_s  ( = swapped Y_k directly) ----
        rcp = big.tile([P, D], f32, name="rcp")
        nc.vector.reciprocal_approx_fast(out=rcp[:], in_=den[:])
        Gts = {}
        for k in range(0, deg + 1):
            if k + 1 not in Pws:
                # the highest swapped power, needed only by the last Y
                Ptop = big.tile([P, D], f32, name=f"P{k + 1}s")
                nc.vector.scalar_tensor_tensor(
                    out=Ptop[:], in0=Pws[k][:], scalar=1.0, in1=Xs[:],
                    op0=MULT, op1=MULT,
                )
                Pws[k + 1] = Ptop
            q = scr.tile([P, D], f32, name=f"q{k}", tag="q")
            Yk = small.tile([P, 1], f32, name=f"Y{k}t")
            nc.vector.scalar_tensor_tensor(
                out=q[:], in0=Pws[k + 1][:], scalar=1.0, in1=rcp[:],
                op0=MULT, op1=MULT, accum_out=Yk[:],
            )
            # g_k = cout * c_k * Y_ks (tiny ACT copy, fired per column)
            Gts[k] = small.tile([P, 1], f32, name=f"G{k}t")
            nc.scalar.mul(Gts[k][:], coefs[:, deg + 1 + k : deg + 2 + k], Yk[:])

        # ---- OUT = swap(X) + g_0 + sum_k g_k X^k ----
        uA = scr.tile([P, D], f32, name="uA", tag="t")
        nc.scalar.activation(uA[:], X[:], AF.Identity,
                             bias=Gts[0][:], scale=Gts[1][:])
        # remaining terms + final join + DMA, split by free-dim halves so
        # the first output DMAs fire while the second half computes
        OUT = big.tile([P, D], f32, name="OUT")
        H = D // 2
        dma_eng = [(nc.sync, nc.scalar), (nc.gpsimd, nc.sync)]
        for h, (engA, engB) in enumerate(dma_eng):
            sl = slice(h * H, (h + 1) * H)
            uCh = scr.tile([P, H], f32, name=f"uCh{h}", tag="zh")
            nc.vector.scalar_tensor_tensor(
                out=uCh[:], in0=P2[:, sl], scalar=Gts[2][:], in1=uA[:, sl],
                op0=MULT, op1=ADD,
            )
            zprev = None
            for k in range(3, deg + 1):
                znx = scr.tile([P, H], f32, name=f"z{k}h{h}", tag="zh")
                nc.vector.scalar_tensor_tensor(
                    out=znx[:], in0=Pw[k][:, sl], scalar=Gts[k][:],
                    in1=(Xs[:, sl] if zprev is None else zprev[:]),
                    op0=MULT, op1=ADD,
                )
                zprev = znx
            zsl = Xs[:, sl] if zprev is None else zprev[:]
            nc.vector.tensor_tensor(out=OUT[:, sl], in0=uCh[:],
                                    in1=zsl, op=ADD)
            engA.dma_start(ov_d[:, sl], OUT[BC:P, sl])
            engB.dma_start(oe_d[:, sl], OUT[0:BC, sl])

    nc.compile()
    return nc


_PROGRAMS: dict[int, object] = {}


def _get_program(deg: int):
    if deg not in _PROGRAMS:
        _PROGRAMS[deg] = _build_program(deg)
    return _PROGRAMS[deg]


def _host_constants(v, e, w_f, w_g, w_h, w_l, w_m, w_n, deg):
    alpha = float(np.dot(w_g.astype(np.float64), w_f.astype(np.float64)))
    beta = float(np.dot(w_h.astype(np.float64), w_m.astype(np.float64)))
    gamma = float(np.dot(w_l.astype(np.float64), w_n.astype(np.float64)))

    # per-batch bound on |s| = |alpha * e_i * v_j|
    m = abs(alpha) * float(
        (np.abs(e).max(axis=1) * np.abs(v).max(axis=1)).max()
    )
    m = max(m * 1.02, 1e-6)

    cheb = np.polynomial.chebyshev.Chebyshev.interpolate(np.exp, deg, domain=[-m, m])
    q = cheb.convert(kind=np.polynomial.polynomial.Polynomial).coef
    q = np.concatenate([q, np.zeros(deg + 1 - len(q))])
    c = np.array([q[k] * alpha**k for k in range(deg + 1)], dtype=np.float64)

    coefs = np.zeros((P, 2 * deg + 2), dtype=np.float32)
    coefs[:, 0] = c[0] * D
    coefs[:, 1 : deg + 1] = c[1:]
    # g-scale applies at the FINAL (already-swapped) position: the v-half
    # rows of OUT accumulate the e-side output (gamma), e-half beta.
    cout = np.where(np.arange(P) < BC, gamma, beta)
    for k in range(deg + 1):
        coefs[:, deg + 1 + k] = cout * c[k]
    return coefs


def _run(inputs: dict, trace: bool = False):
    v = np.ascontiguousarray(np.asarray(inputs["v_input"], dtype=np.float32))
    e = np.ascontiguousarray(np.asarray(inputs["e_input"], dtype=np.float32))
    assert v.shape == (B, D) and e.shape == (B, D), (v.shape, e.shape)
    ws = {k: np.asarray(inputs[k], dtype=np.float32)
          for k in ("w_f", "w_g", "w_h", "w_l", "w_m", "w_n")}

    coefs = _host_constants(
        v, e, ws["w_f"], ws["w_g"], ws["w_h"], ws["w_l"], ws["w_m"], ws["w_n"], DEG
    )

    nc = _get_program(DEG)
    in_maps = []
    for cidx in range(N_CORES):
        sl = slice(cidx * BC, (cidx + 1) * BC)
        in_maps.append(
            {
                "xv": np.ascontiguousarray(v[sl]),
                "xe": np.ascontiguousarray(e[sl]),
                "coefs": coefs,
            }
        )

    res = run_bass_kernel_spmd(nc, in_maps, list(range(N_CORES)), trace=trace)
    out_v = np.concatenate([res.results[c]["out_v"] for c in range(N_CORES)], axis=0)
    out_e = np.concatenate([res.results[c]["out_e"] for c in range(N_CORES)], axis=0)
    return (out_v, out_e), res


def kernel(**inputs):
    (out_v, out_e), _ = _run(inputs, trace=False)
    return out_v, out_e


# revision 24
# speedup vs baseline: 1.0227x; 1.0227x over previous
"""Trainium2 Bass kernel for the AttentionUnit GNN message-passing block.

Math
----
The nn.Module lifts scalars to `channel` dims with rank-1 weights, so the
whole block collapses to per-batch scalar attention:

    s[b,i,j] = alpha * e[b,i] * v[b,j],     alpha = w_g . w_f
    E = exp(s);  cs[j] = sum_i E[i,j];  rs[i] = sum_j E[i,j]
    out_v = v + beta  * E   @ (v / cs),     beta  = w_h . w_m
    out_e = e + gamma * E^T @ (e / rs),     gamma = w_l . w_n

Since |s| <= m ~ 0.3 (data-dependent, computed at runtime), exp(s) is
replaced by a degree-DEG Chebyshev-interpolated polynomial, which makes E a
rank-(DEG+1) matrix  E = sum_k c_k (e^k)(v^k)^T  that is never materialized:

    den = sum_k c_k A'_k X^k        (cs on the v-half, rs on the e-half)
    Y_k = sum_j X^{k+1} / den       (fused multiply-reduce against 1/den)
    out = swap(X) + sum_k g_k X^k   (g_k = swapped, scaled Y_k)

Layout: pure data parallel over 8 cores, 64 batch rows per core, stacked as
X = [v rows (partitions 0..63); e rows (64..127)] so every op handles both
sides at once. Cross-half swaps of [128,few] scalar blocks use two tiny
SBUF->SBUF DMAs; the final residual add uses a pre-swapped copy of the
input DMA'd at start, so no PE/transpose work is needed anywhere.

The polynomial coefficients depend on the input data, so they are passed as
small input tensors -> the compiled NEFF is input-independent and cached.
"""

import os
from contextlib import ExitStack

import numpy as np

import concourse.bass as bass
import concourse.tile as tile
from concourse import bacc, mybir
from concourse.bass_utils import run_bass_kernel_spmd

B = 512          # batch
D = 512          # dim
N_CORES = 8
BC = B // N_CORES  # 64 batch rows per core
P = 128            # partitions: [v (0..63); e (64..127)]
DEG = int(os.environ.get("ATTN_KERNEL_DEG", "4"))

f32 = mybir.dt.float32
MULT = mybir.AluOpType.mult
ADD = mybir.AluOpType.add
NCOL = 8  # padded column count for the R/Y scalar blocks
AF = mybir.ActivationFunctionType


def _build_program(deg: int):
    """Build + compile the single-core Tile program (same NEFF on all 8 cores)."""
    assert deg in (3, 4), "chains below are written for deg in {3, 4}"
    nc = bacc.Bacc(
        "TRN2",
        target_bir_lowering=False,
        debug=False,
        enable_asserts=False,
    )

    xv_d = nc.dram_tensor("xv", [BC, D], f32, kind="ExternalInput")
    xe_d = nc.dram_tensor("xe", [BC, D], f32, kind="ExternalInput")
    # coefs columns: [0] = c_0 * D
    #                [1 : deg+1]       = c_k (k=1..deg)            (den scale)
    #                [deg+1 : 2deg+2]  = swapped-cout * c_k (k=0..deg)
    coefs_d = nc.dram_tensor("coefs", [P, 2 * deg + 2], f32, kind="ExternalInput")
    ov_d = nc.dram_tensor("out_v", [BC, D], f32, kind="ExternalOutput")
    oe_d = nc.dram_tensor("out_e", [BC, D], f32, kind="ExternalOutput")

    with tile.TileContext(nc) as tc, ExitStack() as ctx:
        big = ctx.enter_context(tc.tile_pool(name="big", bufs=1))
        scr = ctx.enter_context(tc.tile_pool(name="scr", bufs=2))
        small = ctx.enter_context(tc.tile_pool(name="small", bufs=1))

        # ---- inputs: X and its half-swapped copy (for the final residual) --
        X = big.tile([P, D], f32, name="X")
        nc.sync.dma_start(X[0:BC, :], xv_d[:])
        nc.scalar.dma_start(X[BC:P, :], xe_d[:])
        Xs = big.tile([P, D], f32, name="Xs")
        nc.sync.dma_start(Xs[BC:P, :], xv_d[:])
        nc.scalar.dma_start(Xs[0:BC, :], xe_d[:])
        coefs = small.tile([P, 2 * deg + 2], f32, name="coefs_t")
        nc.gpsimd.dma_start(coefs[:], coefs_d[:])

        # ---- dual power chains: P_k = X^k (for the output sum) and
        # Ps_k = Xs^k (for the swapped denominator + Y reductions). The
        # row-sums R_k of the X powers are exactly the coefficients the
        # SWAPPED denominator needs, so no cross-half moves are required. --
        R1t = small.tile([P, 1], f32, name="R1t")
        nc.vector.tensor_reduce(R1t[:], X[:], axis=mybir.AxisListType.X, op=ADD)
        R2t = small.tile([P, 1], f32, name="R2t")
        P2 = big.tile([P, D], f32, name="P2")
        nc.scalar.activation(P2[:], X[:], AF.Square, accum_out=R2t[:])
        P2s = big.tile([P, D], f32, name="P2s")
        nc.scalar.activation(P2s[:], Xs[:], AF.Square)
        R3t = small.tile([P, 1], f32, name="R3t")
        P3 = big.tile([P, D], f32, name="P3")
        nc.vector.scalar_tensor_tensor(
            out=P3[:], in0=P2[:], scalar=1.0, in1=X[:],
            op0=MULT, op1=MULT, accum_out=R3t[:],
        )
        Rts = {1: R1t, 2: R2t, 3: R3t}
        Pw = {1: X, 2: P2, 3: P3}
        if deg >= 4:
            R4t = small.tile([P, 1], f32, name="R4t")
            P4 = big.tile([P, D], f32, name="P4")
            nc.scalar.activation(P4[:], P2[:], AF.Square, accum_out=R4t[:])
            Rts[4] = R4t
            Pw[4] = P4
        P3s = big.tile([P, D], f32, name="P3s")
        nc.vector.scalar_tensor_tensor(
            out=P3s[:], in0=P2s[:], scalar=1.0, in1=Xs[:], op0=MULT, op1=MULT,
        )
        Pws = {1: Xs, 2: P2s, 3: P3s}

        # b_k = c_k * R_k (per-column on GpSimd, unblocking as each R lands)
        Bts = {}
        for k in range(1, deg + 1):
            Bts[k] = small.tile([P, 1], f32, name=f"B{k}t")
            nc.gpsimd.tensor_tensor(
                out=Bts[k][:], in0=Rts[k][:], in1=coefs[:, k : k + 1], op=MULT,
            )

        # ---- den_s = swap(den) = cd0 + sum_k b_k Xs^k ----
        dB = scr.tile([P, D], f32, name="dB", tag="t")
        nc.scalar.activation(dB[:], P2s[:], AF.Identity,
                             bias=coefs[:, 0:1], scale=Bts[2][:])
        dA = scr.tile([P, D], f32, name="dA", tag="u")
        nc.vector.scalar_tensor_tensor(
            out=dA[:], in0=Xs[:], scalar=Bts[1][:], in1=dB[:],
            op0=MULT, op1=ADD,
        )
        def emit_p4s():
            P4s = big.tile([P, D], f32, name="P4s")
            nc.scalar.activation(P4s[:], P2s[:], AF.Square)
            Pws[4] = P4s

        if deg >= 4:
            emit_p4s()
        dprev = dA
        for k in range(3, deg + 1):
            dnx = scr.tile([P, D], f32, name=f"d{k}", tag="t" if k % 2 else "u")
            nc.vector.scalar_tensor_tensor(
                out=dnx[:], in0=Pws[k][:], scalar=Bts[k][:], in1=dprev[:],
                op0=MULT, op1=ADD,
            )
            dprev = dnx
        den = dprev
        if deg == 3:
            # emitted late so the in-order ACT stream runs the B/dB ops
            # first (P4s is only consumed by the last Y reduction)
            emit_p4s()

        # ---- Y_ks = sum_j Xs^{k+1} / den_s  ( = swapped Y_k directly) ----
        rcp = big.tile([P, D], f32, name="rcp")
        nc.vector.reciprocal_approx_fast(out=rcp[:], in_=den[:])
        Gts = {}
        for k in range(0, deg + 1):
            if k + 1 not in Pws:
                # the highest swapped power, needed only by the last Y
                Ptop = big.tile([P, D], f32, name=f"P{k + 1}s")
                nc.vector.scalar_tensor_tensor(
                    out=Ptop[:], in0=Pws[k][:], scalar=1.0, in1=Xs[:],
                    op0=MULT, op1=MULT,
                )
                Pws[k + 1] = Ptop
            q = scr.tile([P, D], f32, name=f"q{k}", tag="q")
            Yk = small.tile([P, 1], f32, name=f"Y{k}t")
            nc.vector.scalar_tensor_tensor(
                out=q[:], in0=Pws[k + 1][:], scalar=1.0, in1=rcp[:],
                op0=MULT, op1=MULT, accum_out=Yk[:],
            )
            # g_k = cout * c_k * Y_ks (per-column on GpSimd)
            Gts[k] = small.tile([P, 1], f32, name=f"G{k}t")
            nc.gpsimd.tensor_tensor(
                out=Gts[k][:], in0=Yk[:],
                in1=coefs[:, deg + 1 + k : deg + 2 + k], op=MULT,
            )

        # ---- OUT = swap(X) + g_0 + sum_k g_k X^k ----
        uA = scr.tile([P, D], f32, name="uA", tag="t")
        nc.scalar.activation(uA[:], X[:], AF.Identity,
                             bias=Gts[0][:], scale=Gts[1][:])
        # remaining terms + final join + DMA, split by free-dim halves so
        # the first output DMAs fire while the second half computes
        OUT = big.tile([P, D], f32, name="OUT")
        H = D // 2
        dma_eng = [(nc.sync, nc.scalar), (nc.gpsimd, nc.sync)]
        for h, (engA, engB) in enumerate(dma_eng):
            sl = slice(h * H, (h + 1) * H)
            uCh = scr.tile([P, H], f32, name=f"uCh{h}", tag="zh")
            nc.vector.scalar_tensor_tensor(
                out=uCh[:], in0=P2[:, sl], scalar=Gts[2][:], in1=uA[:, sl],
                op0=MULT, op1=ADD,
            )
            zprev = None
            for k in range(3, deg + 1):
                znx = scr.tile([P, H], f32, name=f"z{k}h{h}", tag="zh")
                nc.vector.scalar_tensor_tensor(
                    out=znx[:], in0=Pw[k][:, sl], scalar=Gts[k][:],
                    in1=(Xs[:, sl] if zprev is None else zprev[:]),
                    op0=MULT, op1=ADD,
                )
                zprev = znx
            zsl = Xs[:, sl] if zprev is None else zprev[:]
            nc.vector.tensor_tensor(out=OUT[:, sl], in0=uCh[:],
                                    in1=zsl, op=ADD)
            engA.dma_start(ov_d[:, sl], OUT[BC:P, sl])
            engB.dma_start(oe_d[:, sl], OUT[0:BC, sl])

    nc.compile()
    return nc


_PROGRAMS: dict[int, object] = {}


def _get_program(deg: int):
    if deg not in _PROGRAMS:
        _PROGRAMS[deg] = _build_program(deg)
    return _PROGRAMS[deg]


def _host_constants(v, e, w_f, w_g, w_h, w_l, w_m, w_n, deg):
    alpha = float(np.dot(w_g.astype(np.float64), w_f.astype(np.float64)))
    beta = float(np.dot(w_h.astype(np.float64), w_m.astype(np.float64)))
    gamma = float(np.dot(w_l.astype(np.float64), w_n.astype(np.float64)))

    # per-batch bound on |s| = |alpha * e_i * v_j|
    m = abs(alpha) * float(
        (np.abs(e).max(axis=1) * np.abs(v).max(axis=1)).max()
    )
    m = max(m * 1.02, 1e-6)

    cheb = np.polynomial.chebyshev.Chebyshev.interpolate(np.exp, deg, domain=[-m, m])
    q = cheb.convert(kind=np.polynomial.polynomial.Polynomial).coef
    q = np.concatenate([q, np.zeros(deg + 1 - len(q))])
    c = np.array([q[k] * alpha**k for k in range(deg + 1)], dtype=np.float64)

    coefs = np.zeros((P, 2 * deg + 2), dtype=np.float32)
    coefs[:, 0] = c[0] * D
    coefs[:, 1 : deg + 1] = c[1:]
    # g-scale applies at the FINAL (already-swapped) position: the v-half
    # rows of OUT accumulate the e-side output (gamma), e-half beta.
    cout = np.where(np.arange(P) < BC, gamma, beta)
    for k in range(deg + 1):
        coefs[:, deg + 1 + k] = cout * c[k]
    return coefs


def _run(inputs: dict, trace: bool = False):
    v = np.ascontiguousarray(np.asarray(inputs["v_input"], dtype=np.float32))
    e = np.ascontiguousarray(np.asarray(inputs["e_input"], dtype=np.float32))
    assert v.shape == (B, D) and e.shape == (B, D), (v.shape, e.shape)
    ws = {k: np.asarray(inputs[k], dtype=np.float32)
          for k in ("w_f", "w_g", "w_h", "w_l", "w_m", "w_n")}

    coefs = _host_constants(
        v, e, ws["w_f"], ws["w_g"], ws["w_h"], ws["w_l"], ws["w_m"], ws["w_n"], DEG
    )

    nc = _get_program(DEG)
    in_maps = []
    for cidx in range(N_CORES):
        sl = slice(cidx * BC, (cidx + 1) * BC)
        in_maps.append(
            {
                "xv": np.ascontiguousarray(v[sl]),
                "xe": np.ascontiguousarray(e[sl]),
                "coefs": coefs,
            }
        )

    res = run_bass_kernel_spmd(nc, in_maps, list(range(N_CORES)), trace=trace)
    out_v = np.concatenate([res.results[c]["out_v"] for c in range(N_CORES)], axis=0)
    out_e = np.concatenate([res.results[c]["out_e"] for c in range(N_CORES)], axis=0)
    return (out_v, out_e), res


def kernel(**inputs):
    (out_v, out_e), _ = _run(inputs, trace=False)
    return out_v, out_e
